# revision 1
# baseline (speedup 1.0000x reference)
"""ColorHistogramLoss Trainium2 kernel.

Problem: loss = mean(|hist(input) - hist(target)|) with 64-bin histograms
per (batch, channel) over [-1, 1), inputs [32, 3, 512, 512] f32.

Strategy (8 cores, data-parallel over batch, 4 batches/core):
  - Binning: w = bf16_rne(v*(63/128) + (191/128 - 2^-8)). The -2^-8 pre-bias
    turns bf16 round-to-nearest into floor onto the 2^-7 grid of [1,2), so
    (w >= 1 + j/64) reproduces searchsorted binning exactly (boundary-rounding
    differences ~1e-5 of elements, loss rel-err ~1e-4).
  - CDF counts per edge j, fused compare+accumulate, split across engines:
      DVE: tensor_scalar(is_ge, imm 1+j/64, accum_out) on bf16 w (4x mode)
      ACT: activation(Sign, bias=-(1+j/64-2^-9), accum_out) on w, in parallel
    Host differentiates the CDF and does the tiny final reduction.
  - Layout: 24 channel-images per core (4 batches x 3 ch x 2 tensors),
    packed 4 per SBUF tile as [128, 8192] f32 -> 6 group tiles.
"""

import numpy as np

BINS = 64
N_CORES = 8
B, C, H, W = 32, 3, 512, 512
NPIX = H * W                  # 262144 per channel-image
B_LOC = B // N_CORES          # 4
IMGS = 2 * B_LOC * C          # 24 channel-images per core
PACK = 4                      # channel-images per SBUF group tile
GROUPS = IMGS // PACK         # 6
PART_PER_IMG = 128 // PACK    # 32 partitions per image
FD = NPIX // PART_PER_IMG     # 8192 free-dim elements per partition

SCALE = float(np.float32(63.0 / 128.0))              # exact in f32
BIAS2 = float(np.float32(191.0 / 128.0) - np.float32(2.0 ** -8))

# edges j=1..63; ACT (Sign) takes the first N_ACT, DVE (is_ge) the rest.
N_ACT = 16

_cache = {}


def _build():
    from concourse import bacc
    import concourse.mybir as mybir
    from concourse.tile import TileContext

    f32 = mybir.dt.float32
    bf16 = mybir.dt.bfloat16

    nc = bacc.Bacc("TRN2", target_bir_lowering=False, debug=False,
                   num_devices=N_CORES)
    x = nc.declare_dram_parameter("x", [GROUPS, 128, FD], f32, isOutput=False)
    bias_a = nc.declare_dram_parameter(
        "bias_a", [128, max(N_ACT, 1)], f32, isOutput=False)
    # counts_d[g, p, j] = #{w >= 1 + j/64} for DVE-owned j   (col 0 unused)
    # counts_a[g, p, j] = sum(sign(w - (1 + j/64 - 2^-9))) for ACT-owned j
    counts_d = nc.declare_dram_parameter(
        "counts_d", [GROUPS, 128, BINS], f32, isOutput=True)
    counts_a = nc.declare_dram_parameter(
        "counts_a", [GROUPS, 128, BINS], f32, isOutput=True)

    edges_act = list(range(1, 1 + N_ACT))
    edges_dve = list(range(1 + N_ACT, BINS))

    with TileContext(nc) as tc, tc.tile_pool(name="p", bufs=2) as pool:
        bt = pool.tile([128, max(N_ACT, 1)], f32, tag="bt")
        nc.sync.dma_start(out=bt[:], in_=bias_a[:])
        for g in range(GROUPS):
            vt = pool.tile([128, FD], f32, tag="vt")
            nc.sync.dma_start(out=vt[:], in_=x[g])
            # w = bf16(v*SCALE + BIAS2): one fused DVE op, f32 2x mode
            w = pool.tile([128, FD], bf16, tag="w")
            nc.vector.tensor_scalar(
                out=w[:], in0=vt[:], scalar1=SCALE, scalar2=BIAS2,
                op0=mybir.AluOpType.mult, op1=mybir.AluOpType.add)

            cnt_d = pool.tile([128, BINS], f32, tag="cntd")
            cnt_a = pool.tile([128, BINS], f32, tag="cnta")
            nc.vector.memset(cnt_d[:], 0.0)
            nc.gpsimd.memset(cnt_a[:], 0.0)
            mask_d = pool.tile([128, FD], bf16, tag="maskd")
            mask_a = pool.tile([128, FD], bf16, tag="maska")
            for j in edges_dve:
                nc.vector.tensor_scalar(
                    out=mask_d[:], in0=w[:], scalar1=float(1.0 + j / 64.0),
                    scalar2=None,
                    op0=mybir.AluOpType.is_ge, op1=mybir.AluOpType.add,
                    accum_out=cnt_d[:, j:j + 1])
            for k, j in enumerate(edges_act):
                nc.scalar.activation(
                    out=mask_a[:], in_=w[:],
                    func=mybir.ActivationFunctionType.Sign,
                    bias=bt[:, k:k + 1], scale=1.0,
                    accum_out=cnt_a[:, j:j + 1])
            nc.sync.dma_start(out=counts_d[g], in_=cnt_d[:])
            nc.sync.dma_start(out=counts_a[g], in_=cnt_a[:])
    nc.finalize()
    return nc


def _get_nc():
    if "nc" not in _cache:
        _cache["nc"] = _build()
    return _cache["nc"]


def _pack_core(inp_c: np.ndarray, tgt_c: np.ndarray) -> np.ndarray:
    """[4,3,512,512] x2 f32 -> [GROUPS, 128, FD]; image i = t*12 + b*3 + c."""
    imgs = np.concatenate(
        [inp_c.reshape(B_LOC * C, NPIX), tgt_c.reshape(B_LOC * C, NPIX)], axis=0)
    return np.ascontiguousarray(
        imgs.reshape(GROUPS, PACK, PART_PER_IMG, FD).reshape(GROUPS, 128, FD))


def _counts_to_loss(results) -> np.float32:
    """results: list of 8 dicts with counts_d/counts_a [GROUPS, 128, BINS]."""
    total = np.float64(0.0)
    for c in range(N_CORES):
        cd = np.asarray(results[c]["counts_d"], np.float64)
        ca = np.asarray(results[c]["counts_a"], np.float64)
        cd = cd.reshape(GROUPS, PACK, PART_PER_IMG, BINS).sum(axis=2)
        ca = ca.reshape(GROUPS, PACK, PART_PER_IMG, BINS).sum(axis=2)
        cdf = np.zeros((IMGS, BINS), np.float64)
        cdf[:, 0] = NPIX
        flat_d = cd.reshape(IMGS, BINS)
        flat_a = ca.reshape(IMGS, BINS)
        for j in range(1, BINS):
            if j <= N_ACT:
                cdf[:, j] = (NPIX + flat_a[:, j]) / 2.0   # sign-sum -> count_ge
            else:
                cdf[:, j] = flat_d[:, j]
        counts = np.empty((IMGS, BINS), np.float64)
        counts[:, :-1] = cdf[:, :-1] - cdf[:, 1:]
        counts[:, -1] = cdf[:, -1]
        hist = counts / NPIX   # [24, 64]; images 0..11 = input, 12..23 = target
        h_in = hist[: B_LOC * C].reshape(B_LOC, C * BINS)
        h_tg = hist[B_LOC * C:].reshape(B_LOC, C * BINS)
        total += np.abs(h_in - h_tg).sum()
    return np.float32(total / (B * C * BINS))


def _bias_np() -> np.ndarray:
    cols = [-(float(np.float32(1.0 + j / 64.0)) - 2.0 ** -9)
            for j in range(1, 1 + N_ACT)] or [0.0]
    return np.tile(np.array(cols, np.float32), (128, 1))


def _make_in_maps(input: np.ndarray, target: np.ndarray):
    inp = np.asarray(input, np.float32)
    tgt = np.asarray(target, np.float32)
    bias = _bias_np()
    in_maps = []
    for c in range(N_CORES):
        sl = slice(c * B_LOC, (c + 1) * B_LOC)
        in_maps.append({"x": _pack_core(inp[sl], tgt[sl]), "bias_a": bias})
    return in_maps


def kernel(input: np.ndarray, target: np.ndarray) -> np.ndarray:
    from concourse.bass_utils import run_bass_kernel_spmd

    nc = _get_nc()
    res = run_bass_kernel_spmd(
        nc, _make_in_maps(input, target), core_ids=list(range(N_CORES)))
    return np.asarray(_counts_to_loss(res.results), np.float32)



# revision 2
# speedup vs baseline: 16.7526x; 16.7526x over previous
"""ColorHistogramLoss Trainium2 kernel.

Problem: loss = mean(|hist(input) - hist(target)|) with 64-bin histograms
per (batch, channel) over [-1, 1), inputs [32, 3, 512, 512] f32.

Strategy (8 cores, data-parallel over batch, 4 batches/core):
  - Binning: w = bf16_rne(v*(63/128) + (191/128 - 2^-8)). The -2^-8 pre-bias
    turns bf16 round-to-nearest into floor onto the 2^-7 grid of [1,2), so
    (w >= 1 + j/64) reproduces searchsorted binning exactly (boundary-rounding
    differences ~1e-5 of elements, loss rel-err ~1e-4).
  - CDF counts per edge j, fused compare+accumulate, split across engines:
      DVE: tensor_scalar(is_ge, imm 1+j/64, accum_out) on bf16 w (4x mode)
      ACT: activation(Sign, bias=-(1+j/64-2^-9), accum_out) on w, in parallel
    Host differentiates the CDF and does the tiny final reduction.
  - w-build (affine f32->bf16) runs on GPSIMD to keep DVE/ACT free for edges.
  - Layout: 24 channel-images per core (4 batches x 3 ch x 2 tensors),
    packed 4 per SBUF tile as [128, 8192] f32 -> 6 group tiles.
"""

import numpy as np

BINS = 64
N_CORES = 8
B, C, H, W = 32, 3, 512, 512
NPIX = H * W                  # 262144 per channel-image
B_LOC = B // N_CORES          # 4
IMGS = 2 * B_LOC * C          # 24 channel-images per core
PACK = 4                      # channel-images per SBUF group tile
GROUPS = IMGS // PACK         # 6
PART_PER_IMG = 128 // PACK    # 32 partitions per image
FD = NPIX // PART_PER_IMG     # 8192 free-dim elements per partition

SCALE = float(np.float32(63.0 / 128.0))              # exact in f32
BIAS2 = float(np.float32(191.0 / 128.0) - np.float32(2.0 ** -8))

# edges j=1..63; ACT (Sign) takes the first N_ACT, DVE (is_ge) the rest.
N_ACT = 15
# engine for the f32->bf16 affine build: "gpsimd" | "vector"
W_BUILD = "gpsimd"

_cache = {}


def _build(reps=1, n_act=N_ACT, w_build=W_BUILD):
    from concourse import bacc
    import concourse.mybir as mybir
    from concourse.tile import TileContext

    f32 = mybir.dt.float32
    bf16 = mybir.dt.bfloat16

    nc = bacc.Bacc("TRN2", target_bir_lowering=False, debug=False,
                   num_devices=N_CORES)
    x = nc.declare_dram_parameter("x", [GROUPS, 128, FD], f32, isOutput=False)
    bias_a = nc.declare_dram_parameter(
        "bias_a", [128, max(n_act, 1)], f32, isOutput=False)
    # counts_d[g, p, j] = #{w >= 1 + j/64} for DVE-owned j   (col 0 unused)
    # counts_a[g, p, j] = sum(sign(w - (1 + j/64 - 2^-9))) for ACT-owned j
    counts_d = nc.declare_dram_parameter(
        "counts_d", [GROUPS, 128, BINS], f32, isOutput=True)
    counts_a = nc.declare_dram_parameter(
        "counts_a", [GROUPS, 128, BINS], f32, isOutput=True)

    edges_act = list(range(1, 1 + n_act))
    edges_dve = list(range(1 + n_act, BINS))

    with TileContext(nc) as tc, tc.tile_pool(name="p", bufs=2) as pool:
        bt = pool.tile([128, max(n_act, 1)], f32, tag="bt")
        nc.sync.dma_start(out=bt[:], in_=bias_a[:])
        for rep in range(reps):
            for g in range(GROUPS):
                vt = pool.tile([128, FD], f32, tag="vt")
                nc.sync.dma_start(out=vt[:], in_=x[g])
                # w = bf16(v*SCALE + BIAS2): one fused op
                w = pool.tile([128, FD], bf16, tag="w")
                beng = nc.gpsimd if w_build == "gpsimd" else nc.vector
                beng.tensor_scalar(
                    out=w[:], in0=vt[:], scalar1=SCALE, scalar2=BIAS2,
                    op0=mybir.AluOpType.mult, op1=mybir.AluOpType.add)

                cnt_d = pool.tile([128, BINS], f32, tag="cntd")
                cnt_a = pool.tile([128, BINS], f32, tag="cnta")
                nc.vector.memset(cnt_d[:], 0.0)
                nc.gpsimd.memset(cnt_a[:], 0.0)
                mask_d = pool.tile([128, FD], bf16, tag="maskd")
                mask_a = pool.tile([128, FD], bf16, tag="maska")
                for j in edges_dve:
                    nc.vector.tensor_scalar(
                        out=mask_d[:], in0=w[:], scalar1=float(1.0 + j / 64.0),
                        scalar2=None,
                        op0=mybir.AluOpType.is_ge, op1=mybir.AluOpType.add,
                        accum_out=cnt_d[:, j:j + 1])
                for k, j in enumerate(edges_act):
                    nc.scalar.activation(
                        out=mask_a[:], in_=w[:],
                        func=mybir.ActivationFunctionType.Sign,
                        bias=bt[:, k:k + 1], scale=1.0,
                        accum_out=cnt_a[:, j:j + 1])
                nc.sync.dma_start(out=counts_d[g], in_=cnt_d[:])
                nc.sync.dma_start(out=counts_a[g], in_=cnt_a[:])
    nc.finalize()
    return nc


def _get_nc(reps=1):
    key = ("nc", reps, N_ACT, W_BUILD)
    if key not in _cache:
        _cache[key] = _build(reps=reps)
    return _cache[key]


def _pack_core(inp_c: np.ndarray, tgt_c: np.ndarray) -> np.ndarray:
    """[4,3,512,512] x2 f32 -> [GROUPS, 128, FD]; image i = t*12 + b*3 + c."""
    imgs = np.concatenate(
        [inp_c.reshape(B_LOC * C, NPIX), tgt_c.reshape(B_LOC * C, NPIX)], axis=0)
    return np.ascontiguousarray(
        imgs.reshape(GROUPS, PACK, PART_PER_IMG, FD).reshape(GROUPS, 128, FD))


def _counts_to_loss(results, n_act=N_ACT) -> np.float32:
    """results: list of 8 dicts with counts_d/counts_a [GROUPS, 128, BINS]."""
    total = np.float64(0.0)
    for c in range(N_CORES):
        cd = np.asarray(results[c]["counts_d"], np.float64)
        ca = np.asarray(results[c]["counts_a"], np.float64)
        cd = cd.reshape(GROUPS, PACK, PART_PER_IMG, BINS).sum(axis=2)
        ca = ca.reshape(GROUPS, PACK, PART_PER_IMG, BINS).sum(axis=2)
        cdf = np.zeros((IMGS, BINS), np.float64)
        cdf[:, 0] = NPIX
        flat_d = cd.reshape(IMGS, BINS)
        flat_a = ca.reshape(IMGS, BINS)
        for j in range(1, BINS):
            if j <= n_act:
                cdf[:, j] = (NPIX + flat_a[:, j]) / 2.0   # sign-sum -> count_ge
            else:
                cdf[:, j] = flat_d[:, j]
        counts = np.empty((IMGS, BINS), np.float64)
        counts[:, :-1] = cdf[:, :-1] - cdf[:, 1:]
        counts[:, -1] = cdf[:, -1]
        hist = counts / NPIX   # [24, 64]; images 0..11 = input, 12..23 = target
        h_in = hist[: B_LOC * C].reshape(B_LOC, C * BINS)
        h_tg = hist[B_LOC * C:].reshape(B_LOC, C * BINS)
        total += np.abs(h_in - h_tg).sum()
    return np.float32(total / (B * C * BINS))


def _bias_np(n_act=N_ACT) -> np.ndarray:
    cols = [-(float(np.float32(1.0 + j / 64.0)) - 2.0 ** -9)
            for j in range(1, 1 + n_act)] or [0.0]
    return np.tile(np.array(cols, np.float32), (128, 1))


def _make_in_maps(input: np.ndarray, target: np.ndarray):
    inp = np.asarray(input, np.float32)
    tgt = np.asarray(target, np.float32)
    bias = _bias_np()
    in_maps = []
    for c in range(N_CORES):
        sl = slice(c * B_LOC, (c + 1) * B_LOC)
        in_maps.append({"x": _pack_core(inp[sl], tgt[sl]), "bias_a": bias})
    return in_maps


def kernel(input: np.ndarray, target: np.ndarray) -> np.ndarray:
    from concourse.bass_utils import run_bass_kernel_spmd

    nc = _get_nc()
    res = run_bass_kernel_spmd(
        nc, _make_in_maps(input, target), core_ids=list(range(N_CORES)))
    return np.asarray(_counts_to_loss(res.results), np.float32)


# revision 6
# speedup vs baseline: 42.0126x; 2.5078x over previous
"""ColorHistogramLoss Trainium2 kernel.

Problem: loss = mean(|hist(input) - hist(target)|) with 64-bin histograms
per (batch, channel) over [-1, 1), inputs [32, 3, 512, 512] f32.

Strategy (8 cores, data-parallel over batch, 4 batches/core):
  - Binning: w = bf16_rne(v*(63/128) + (191/128 - 2^-8)). The -2^-8 pre-bias
    turns bf16 round-to-nearest into floor onto the 2^-7 grid of [1,2), so
    (w >= 1 + j/64) reproduces searchsorted binning exactly.
  - CDF counts per edge j=1..63, three parallel paths (measured rates/pass):
      ACT  (6.8us): activation(Sign, bias) with accum_out   [self-contained]
      PE   (DVE mask 2.3us @4x + PE ones-matmul reduce ~2-3.4us):
           DVE is_ge WITHOUT accum (keeps 4x mode), PE reduces the mask
           along partitions into PSUM via a constant [128,32] selector
           (8 edge-slots x 4 images), 16 chunk-matmuls of N=512 per edge,
           PSUM [32,512] per 8-edge batch -> DMA to DRAM, host sums 512.
      DVE  (8.4us): tensor_scalar is_ge WITH accum_out (reduce uop is 1x)
    DVE-with-accum is 3.8x slower than without: the reduce variant runs at
    1x; that's why masks+PE beat direct accumulation.
  - w-build (affine f32->bf16) runs on GPSIMD; host does final tiny math.
  - Layout: 24 channel-images per core (4 batches x 3 ch x 2 tensors),
    packed 4 per SBUF tile as [128, 8192] f32 -> 6 group tiles.
"""

import numpy as np

BINS = 64
N_CORES = 8
B, C, H, W = 32, 3, 512, 512
NPIX = H * W                  # 262144 per channel-image
B_LOC = B // N_CORES          # 4
IMGS = 2 * B_LOC * C          # 24 channel-images per core
PACK = 4                      # channel-images per SBUF group tile
GROUPS = IMGS // PACK         # 6
PART_PER_IMG = 128 // PACK    # 32 partitions per image
FD = NPIX // PART_PER_IMG     # 8192 free-dim elements per partition

SCALE = float(np.float32(63.0 / 128.0))              # exact in f32
BIAS2 = float(np.float32(191.0 / 128.0) - np.float32(2.0 ** -8))

# edge split: j=1..N_ACT on ACT; next N_PE on the DVE-mask+PE-reduce path;
# the rest (to 63) on DVE with accum.
N_ACT = 20
N_PE = 40                      # multiple of 4 (PSUM batches of 4 edges/banks)
N_DVE = 63 - N_ACT - N_PE
PE_BATCH = 4                   # edges per PSUM eviction batch (1 bank each)
N_BATCH = N_PE // PE_BATCH     # eviction batches per group
MMCH = 16                      # moving chunks per mask (8192/512)
MMN = FD // MMCH               # 512

W_BUILD = "gpsimd"

_cache = {}


def _build(reps=1):
    from concourse import bacc
    import concourse.mybir as mybir
    from concourse.tile import TileContext

    f32 = mybir.dt.float32
    bf16 = mybir.dt.bfloat16

    nc = bacc.Bacc("TRN2", target_bir_lowering=False, debug=False,
                   num_devices=N_CORES)
    x = nc.declare_dram_parameter("x", [GROUPS, 128, FD], f32, isOutput=False)
    bias_a = nc.declare_dram_parameter(
        "bias_a", [128, max(N_ACT, 1)], f32, isOutput=False)
    # sel[p, img] = 1 iff partition p belongs to image img
    sel = nc.declare_dram_parameter("sel", [128, 4], bf16, isOutput=False)
    counts_a = nc.declare_dram_parameter(
        "counts_a", [GROUPS, 128, BINS], f32, isOutput=True)
    counts_d = nc.declare_dram_parameter(
        "counts_d", [GROUPS, 128, BINS], f32, isOutput=True)
    # counts_p[g, b, img, m*512+f] = partial count (over partitions) of
    # edge (N_ACT + b*4 + m + 1) for image img of group g, moving col f
    counts_p = nc.declare_dram_parameter(
        "counts_p", [GROUPS, N_BATCH, 4, PE_BATCH * MMN], f32, isOutput=True)

    edges_act = list(range(1, 1 + N_ACT))
    edges_pe = list(range(1 + N_ACT, 1 + N_ACT + N_PE))
    edges_dve = list(range(1 + N_ACT + N_PE, BINS))

    with TileContext(nc) as tc, \
            tc.tile_pool(name="p", bufs=2) as pool, \
            tc.tile_pool(name="ps", bufs=2, space="PSUM") as psum_pool:
        bt = pool.tile([128, max(N_ACT, 1)], f32, tag="bt")
        nc.sync.dma_start(out=bt[:], in_=bias_a[:])
        selt = pool.tile([128, 4], bf16, tag="sel")
        nc.sync.dma_start(out=selt[:], in_=sel[:])
        for rep in range(reps):
            for g in range(GROUPS):
                w = pool.tile([128, FD], bf16, tag="w")
                beng = nc.gpsimd if W_BUILD == "gpsimd" else nc.vector
                HF = FD // 2
                for h in range(2):
                    vt = pool.tile([128, HF], f32, tag="vt")
                    nc.sync.dma_start(out=vt[:], in_=x[g][:, h * HF:(h + 1) * HF])
                    beng.tensor_scalar(
                        out=w[:, h * HF:(h + 1) * HF], in0=vt[:],
                        scalar1=SCALE, scalar2=BIAS2,
                        op0=mybir.AluOpType.mult, op1=mybir.AluOpType.add)

                cnt_a = pool.tile([128, BINS], f32, tag="cnta")
                cnt_d = pool.tile([128, BINS], f32, tag="cntd")
                nc.vector.memset(cnt_d[:], 0.0)
                nc.vector.memset(cnt_a[:], 0.0)
                mask_a = pool.tile([128, FD], bf16, tag="maska")

                # PE path: per edge accumulate [4, 512] in one PSUM bank
                # (out rows 0..3 = images, stationary = [128,4] selector);
                # DVE copies PSUM->SBUF staging, one DMA per 4-edge batch.
                for b in range(N_BATCH):
                    stage = pool.tile([4, PE_BATCH * MMN], f32, tag="stage")
                    for m in range(PE_BATCH):
                        j = edges_pe[b * PE_BATCH + m]
                        mask = pool.tile([128, FD], bf16, tag="maskp")
                        nc.vector.tensor_scalar(
                            out=mask[:], in0=w[:],
                            scalar1=float(1.0 + j / 64.0), scalar2=None,
                            op0=mybir.AluOpType.is_ge)
                        ps = psum_pool.tile([128, MMN], f32, tag="ps")
                        for c in range(MMCH):
                            nc.tensor.matmul(
                                out=ps[0:4, :],
                                lhsT=selt[:],
                                rhs=mask[:, c * MMN:(c + 1) * MMN],
                                start=(c == 0),
                                stop=(c == MMCH - 1),
                            )
                        nc.vector.tensor_copy(
                            out=stage[:, m * MMN:(m + 1) * MMN], in_=ps[0:4, :])
                    nc.sync.dma_start(out=counts_p[g, b], in_=stage[:])

                # ACT path
                for k, j in enumerate(edges_act):
                    nc.scalar.activation(
                        out=mask_a[:], in_=w[:],
                        func=mybir.ActivationFunctionType.Sign,
                        bias=bt[:, k:k + 1], scale=1.0,
                        accum_out=cnt_a[:, j:j + 1])

                # DVE-with-accum path (few edges)
                mask_d = pool.tile([128, FD], bf16, tag="maskd")
                for j in edges_dve:
                    nc.vector.tensor_scalar(
                        out=mask_d[:], in0=w[:],
                        scalar1=float(1.0 + j / 64.0), scalar2=None,
                        op0=mybir.AluOpType.is_ge, op1=mybir.AluOpType.add,
                        accum_out=cnt_d[:, j:j + 1])

                nc.sync.dma_start(out=counts_a[g], in_=cnt_a[:])
                nc.sync.dma_start(out=counts_d[g], in_=cnt_d[:])
    nc.finalize()
    return nc


def _get_nc(reps=1):
    key = ("nc", reps, N_ACT, N_PE)
    if key not in _cache:
        _cache[key] = _build(reps=reps)
    return _cache[key]


def _pack_core(inp_c: np.ndarray, tgt_c: np.ndarray) -> np.ndarray:
    """[4,3,512,512] x2 f32 -> [GROUPS, 128, FD]; image i = t*12 + b*3 + c."""
    imgs = np.concatenate(
        [inp_c.reshape(B_LOC * C, NPIX), tgt_c.reshape(B_LOC * C, NPIX)], axis=0)
    return np.ascontiguousarray(
        imgs.reshape(GROUPS, PACK, PART_PER_IMG, FD).reshape(GROUPS, 128, FD))


def _sel_np() -> np.ndarray:
    """sel[p, img] = 1 iff p in [img*32, (img+1)*32)."""
    s = np.zeros((128, 4), np.float32)
    for img in range(4):
        s[img * 32:(img + 1) * 32, img] = 1.0
    from ml_dtypes import bfloat16
    return s.astype(bfloat16)


def _counts_to_loss(results) -> np.float32:
    """results: list of 8 dicts with counts_a/counts_d/counts_p."""
    total = np.float64(0.0)
    for c in range(N_CORES):
        ca = np.asarray(results[c]["counts_a"], np.float64)
        cd = np.asarray(results[c]["counts_d"], np.float64)
        cp = np.asarray(results[c]["counts_p"], np.float64)
        ca = ca.reshape(GROUPS, PACK, PART_PER_IMG, BINS).sum(axis=2)
        cd = cd.reshape(GROUPS, PACK, PART_PER_IMG, BINS).sum(axis=2)
        # cp[g, b, img, m*512+f] -> per-(g, img, edge) sums
        cps = cp.reshape(GROUPS, N_BATCH, 4, PE_BATCH, MMN).sum(axis=4)  # [g,b,img,m]
        cdf = np.zeros((GROUPS, PACK, BINS), np.float64)
        cdf[:, :, 0] = NPIX
        for j in range(1, BINS):
            if j <= N_ACT:
                cdf[:, :, j] = (NPIX + ca[:, :, j]) / 2.0
            elif j <= N_ACT + N_PE:
                t = j - 1 - N_ACT
                cdf[:, :, j] = cps[:, t // PE_BATCH, :, t % PE_BATCH]
            else:
                cdf[:, :, j] = cd[:, :, j]
        counts = np.empty((GROUPS, PACK, BINS), np.float64)
        counts[:, :, :-1] = cdf[:, :, :-1] - cdf[:, :, 1:]
        counts[:, :, -1] = cdf[:, :, -1]
        hist = (counts / NPIX).reshape(IMGS, BINS)
        h_in = hist[: B_LOC * C].reshape(B_LOC, C * BINS)
        h_tg = hist[B_LOC * C:].reshape(B_LOC, C * BINS)
        total += np.abs(h_in - h_tg).sum()
    return np.float32(total / (B * C * BINS))


def _bias_np() -> np.ndarray:
    cols = [-(float(np.float32(1.0 + j / 64.0)) - 2.0 ** -9)
            for j in range(1, 1 + N_ACT)] or [0.0]
    return np.tile(np.array(cols, np.float32), (128, 1))


def _make_in_maps(input: np.ndarray, target: np.ndarray):
    inp = np.asarray(input, np.float32)
    tgt = np.asarray(target, np.float32)
    bias = _bias_np()
    selm = _sel_np()
    in_maps = []
    for c in range(N_CORES):
        sl = slice(c * B_LOC, (c + 1) * B_LOC)
        in_maps.append({"x": _pack_core(inp[sl], tgt[sl]), "bias_a": bias,
                        "sel": selm})
    return in_maps


def kernel(input: np.ndarray, target: np.ndarray) -> np.ndarray:
    from concourse.bass_utils import run_bass_kernel_spmd

    nc = _get_nc()
    res = run_bass_kernel_spmd(
        nc, _make_in_maps(input, target), core_ids=list(range(N_CORES)))
    return np.asarray(_counts_to_loss(res.results), np.float32)
